# revision 13
# baseline (speedup 1.0000x reference)
"""Bass/Tile TRN2 kernel for nn_BatchGraphAttentionLayer (v2).

Reference computation (per batch b):
    Wh  = h[b] @ W                    # [64, 256]
    s1  = Wh @ a[:256], s2 = Wh @ a[256:]
    e   = leaky_relu(s1[i] + s2[j])   # [64, 64] (i rows, j cols)
    att = softmax over axis i of where(adj[i,j]>0, e, -9e15)
    out = elu(att @ Wh)               # h_prime[i] = sum_j att[i,j] Wh[j]

Sharding: data-parallel over batch, 8 cores x 4 batches.

v2 design (single bf16, measured absmax-rel err ~6e-3 vs 2e-2 gate):
  - a is folded into W on the host: W_ext = [W | W@a1 | W@a2] bf16
    [16384, 258].  The projection matmul then directly produces
    s1, s2 as PSUM columns 256:257 -- no separate score pass.
  - All h/W slab DMAs are issued up front into dedicated SBUF buffers
    (16 MiB total, fits).  No buffer reuse -> no DMA head-of-line
    stalls.  h streams on the sync HWDGE queue, W on the scalar one
    (~8 MiB each).  Slab sizes taper at both ends so the PE starts
    early and finishes shortly after the last byte lands.
  - Attention: batches are stacked 2 per 128-partition tile ("pairs");
    per pair, a rank-2 matmul builds the logit tile
        eT[j, i] = s1^t[i] + s2^t[j]
    from stationary S_t[2,128] = [ones; s2^t] and moving
    V_t[2,128] = [s1^t; ones], both derived from one PE transpose of
    the PSUM score columns via per-partition selector scalars.
  - softmax: exp(leaky_relu(e)) (logits are bounded, |s1+s2| < ~8, so
    unmasked exp is safe), masked by a 0/1 mask multiply, row-sum +
    reciprocal, scale.
  - out = elu(att @ Wh) with att/Wh in bf16 (error contribution
    negligible, keeps the PE in fast bf16 mode).
"""

import os
from contextlib import ExitStack

import ml_dtypes
import numpy as np

import concourse.bass as bass
import concourse.tile as tile
from concourse import bacc, mybir
from concourse.bass_utils import run_bass_kernel_spmd

F32 = mybir.dt.float32
BF16 = mybir.dt.bfloat16

B, N, IN, OUT = 32, 64, 16384, 256
OUTE = OUT + 2               # W_ext columns (W | Wa1 | Wa2)
NCORES = 8
BPC = B // NCORES            # batches per core = 4
M = BPC * N                  # local rows = 256
P = 128
ALPHA = 0.2

KSUB = IN // P               # 128 k-subtiles of 128 rows
# taper both ends: fast PE ramp-up, short post-stream crunch
SLABS = [1, 1, 2, 4, 8, 16, 16, 16, 16, 16, 16, 8, 4, 2, 1, 1]
assert sum(SLABS) == KSUB

_NC = None
LAST_EXEC_NS = None
LAST_RESULTS = None


def _build_kernel(ctx: ExitStack, tc: tile.TileContext, out, h, w, mm256, eye,
                  selc):
    nc = tc.nc

    consts = ctx.enter_context(tc.tile_pool(name="consts", bufs=1))
    hpool = ctx.enter_context(tc.tile_pool(name="hslab", bufs=1))
    wpool = ctx.enter_context(tc.tile_pool(name="wslab", bufs=1))
    whpool = ctx.enter_context(tc.tile_pool(name="wh", bufs=1))
    attp = ctx.enter_context(tc.tile_pool(name="att", bufs=1))
    ps_accp = ctx.enter_context(tc.tile_pool(name="psacc", bufs=1, space="PSUM"))
    ps_smallp = ctx.enter_context(tc.tile_pool(name="pssmall", bufs=1, space="PSUM"))
    ps_ep = ctx.enter_context(tc.tile_pool(name="pse", bufs=1, space="PSUM"))
    ps_op = ctx.enter_context(tc.tile_pool(name="pso", bufs=1, space="PSUM"))

    # ---- constants (gpsimd/SWDGE keeps the HWDGE queues clear) ----
    sb_eye = consts.tile([P, P], F32)
    nc.gpsimd.dma_start(sb_eye, eye)
    sb_mm = consts.tile([P, 2 * P], F32)
    nc.gpsimd.dma_start(sb_mm, mm256)
    # [2,2] selector: col 0 = (1,0) "sel", col 1 = (0,1) "it".  Engine ops
    # can't address partition bases other than 0/32/64/96, so per-pair
    # [2,128] e-matmul operands are built from the transposed score rows
    # with per-partition scalars:  S_t = pst*it + sel = [ones; s2],
    # V_t = pst*sel + it = [s1; ones].
    sb_selc = consts.tile([2, 2], F32)
    nc.gpsimd.dma_start(sb_selc, selc)

    # ---- stream DMAs: all issued up front, dedicated buffers ----
    hs = []
    ws = []
    k0 = 0
    for s, nsub in enumerate(SLABS):
        ksl = slice(k0 * P, (k0 + nsub) * P)
        ht = hpool.tile([P, nsub, M], BF16, tag=f"h{s}", name=f"h{s}")
        nc.sync.dma_start(ht, h[ksl, :].rearrange("(p c) m -> p c m", p=P))
        wt = wpool.tile([P, nsub, OUTE], BF16, tag=f"w{s}", name=f"w{s}")
        nc.scalar.dma_start(wt, w[ksl, :].rearrange("(p c) o -> p c o", p=P))
        hs.append(ht)
        ws.append(wt)
        k0 += nsub

    # ---- projection: Wh_ext accumulated in PSUM (m-major) ----
    ps_wh = [ps_accp.tile([P, OUTE], F32, tag=f"ps_wh{t}", name=f"ps_wh{t}")
             for t in range(2)]
    nslab = len(SLABS)
    for s, nsub in enumerate(SLABS):
        first = (s == 0)
        last = (s == nslab - 1)
        for c in range(nsub):
            for t in range(2):
                nc.tensor.matmul(ps_wh[t],
                                 lhsT=hs[s][:, c, t * P:(t + 1) * P],
                                 rhs=ws[s][:, c, :],
                                 start=(first and c == 0),
                                 stop=(last and c == nsub - 1),
                                 skip_group_check=True)

    # ---- scores -> rows via per-pair PE transposes ----
    # sc[:, 0:2] = (s1^0, s2^0), sc[:, 2:4] = (s1^1, s2^1)
    sc = attp.tile([P, 4], F32, tag="sc")
    nc.vector.tensor_copy(out=sc[:, 0:2], in_=ps_wh[0][:, OUT:OUTE])
    nc.scalar.copy(out=sc[:, 2:4], in_=ps_wh[1][:, OUT:OUTE])
    sel = sb_selc[:, 0:1]
    it = sb_selc[:, 1:2]
    ps_e = [ps_ep.tile([P, P], F32, tag=f"ps_e{t}", name=f"ps_e{t}")
            for t in range(2)]
    for t in range(2):
        pst = ps_smallp.tile([2, P], F32, tag=f"pst{t}", name=f"pst{t}")
        nc.tensor.transpose(pst, sc[:, 2 * t:2 * t + 2], sb_eye)
        st = attp.tile([2, P], BF16, tag=f"st{t}")
        nc.vector.tensor_scalar(st, pst, it, sel,
                                mybir.AluOpType.mult, mybir.AluOpType.add)
        vt = attp.tile([2, P], BF16, tag=f"vt{t}")
        nc.vector.tensor_scalar(vt, pst, sel, it,
                                mybir.AluOpType.mult, mybir.AluOpType.add)
        # eT[j, i] = s1^t[i] + s2^t[j] for this pair's column block
        nc.tensor.matmul(ps_e[t], lhsT=st, rhs=vt, start=True, stop=True)

    # Wh to SBUF in bf16 (moving operand of the output matmul)
    wh_m = [whpool.tile([P, OUT], BF16, tag=f"wh_m{t}", name=f"wh_m{t}")
            for t in range(2)]
    nc.vector.tensor_copy(out=wh_m[0], in_=ps_wh[0][:, :OUT])
    nc.scalar.copy(out=wh_m[1], in_=ps_wh[1][:, :OUT])

    # ---- attention: leaky on DVE per pair, rest on [128, 256] tiles ----
    lk = attp.tile([P, 2 * P], F32, tag="lk")
    for t in range(2):
        vb = attp.tile([P, P], F32, tag=f"vb{t}")
        nc.vector.tensor_scalar_mul(vb, ps_e[t], ALPHA)
        nc.vector.tensor_tensor(lk[:, t * P:(t + 1) * P], ps_e[t], vb,
                                mybir.AluOpType.max)
    pexp = attp.tile([P, 2 * P], F32, tag="pexp")
    nc.scalar.activation(pexp, lk, mybir.ActivationFunctionType.Exp)
    pexpm = attp.tile([P, 2 * P], F32, tag="pexpm")
    nc.vector.tensor_tensor(pexpm, pexp, sb_mm, mybir.AluOpType.mult)
    rs = attp.tile([P, 2], F32, tag="rs")
    nc.vector.tensor_reduce(rs[:, 0:1], pexpm[:, 0:P],
                            axis=mybir.AxisListType.X, op=mybir.AluOpType.add)
    nc.vector.tensor_reduce(rs[:, 1:2], pexpm[:, P:2 * P],
                            axis=mybir.AxisListType.X, op=mybir.AluOpType.add)
    rinv = attp.tile([P, 2], F32, tag="rinv")
    nc.vector.reciprocal(rinv, rs)
    att = attp.tile([P, 2 * P], BF16, tag="attw")
    nc.vector.tensor_scalar_mul(att[:, 0:P], pexpm[:, 0:P], rinv[:, 0:1])
    nc.vector.tensor_scalar_mul(att[:, P:2 * P], pexpm[:, P:2 * P],
                                rinv[:, 1:2])

    # ---- out = elu(att @ Wh), pair-interleaved ----
    for t in range(2):
        ps_o = ps_op.tile([P, OUT], F32, tag=f"ps_o{t}", name=f"ps_o{t}")
        nc.tensor.matmul(ps_o, lhsT=att[:, t * P:(t + 1) * P], rhs=wh_m[t],
                         start=True, stop=True)
        # elu(x) = max(x,0) - 1 + exp(min(x,0))
        m0 = attp.tile([P, OUT], F32, tag=f"m0{t}")
        nc.vector.tensor_scalar_min(m0, ps_o, 0.0)
        ex = attp.tile([P, OUT], F32, tag=f"ex{t}")
        nc.scalar.activation(ex, m0, mybir.ActivationFunctionType.Exp)
        rm1 = attp.tile([P, OUT], F32, tag=f"rm1{t}")
        nc.vector.tensor_scalar(rm1, ps_o, 0.0, -1.0,
                                mybir.AluOpType.max, mybir.AluOpType.add)
        ot = attp.tile([P, OUT], F32, tag=f"ot{t}")
        nc.vector.tensor_tensor(ot, ex, rm1, mybir.AluOpType.add)
        oeng = nc.sync if t == 0 else nc.scalar
        oeng.dma_start(out[t * P:(t + 1) * P, :], ot)


def _get_nc():
    global _NC
    if _NC is not None:
        return _NC
    nc = bacc.Bacc("TRN2", target_bir_lowering=False, debug=False,
                   num_devices=NCORES, disable_frame_to_traceback=True)
    h = nc.dram_tensor("h", [IN, M], BF16, kind="ExternalInput").ap()
    w = nc.dram_tensor("w", [IN, OUTE], BF16, kind="ExternalInput").ap()
    mm256 = nc.dram_tensor("mm256", [P, 2 * P], F32, kind="ExternalInput").ap()
    eye = nc.dram_tensor("eye", [P, P], F32, kind="ExternalInput").ap()
    selc = nc.dram_tensor("selc", [2, 2], F32, kind="ExternalInput").ap()
    out = nc.dram_tensor("out", [M, OUT], F32, kind="ExternalOutput").ap()
    with tile.TileContext(nc) as tc:
        with ExitStack() as ctx:
            _build_kernel(ctx, tc, out, h, w, mm256, eye, selc)
    nc.compile()
    _NC = nc
    return nc


def kernel(h: np.ndarray, adj: np.ndarray, W: np.ndarray, a: np.ndarray
           ) -> np.ndarray:
    global LAST_EXEC_NS, LAST_RESULTS
    h = np.asarray(h, dtype=np.float32)
    W = np.asarray(W, dtype=np.float32)
    a = np.ascontiguousarray(np.asarray(a, dtype=np.float32)).reshape(2 * OUT)
    assert h.shape == (B, N, IN) and W.shape == (IN, OUT)

    nc = _get_nc()

    # mask [j~, i~]: pair-stacked adj^T on the block diagonal, both
    # column halves identical (one per pair)
    adjT = (np.asarray(adj) > 0).T.astype(np.float32)
    mm = np.zeros((P, P), np.float32)
    mm[:N, :N] = adjT
    mm[N:, N:] = adjT
    mm256 = np.ascontiguousarray(np.concatenate([mm, mm], axis=1))
    eye = np.eye(P, dtype=np.float32)
    selc = np.array([[1, 0], [0, 1]], dtype=np.float32)

    Wa1 = (W.astype(np.float64) @ a[:OUT].astype(np.float64)).astype(np.float32)
    Wa2 = (W.astype(np.float64) @ a[OUT:].astype(np.float64)).astype(np.float32)
    w_ext = np.concatenate([W, Wa1[:, None], Wa2[:, None]], axis=1)
    w_ext = np.ascontiguousarray(w_ext).astype(ml_dtypes.bfloat16)

    in_maps = []
    for c in range(NCORES):
        hT = h[c * BPC:(c + 1) * BPC].reshape(M, IN).T
        in_maps.append({
            "h": np.ascontiguousarray(hT).astype(ml_dtypes.bfloat16),
            "w": w_ext, "mm256": mm256, "eye": eye, "selc": selc,
        })

    trace = os.environ.get("GAT_TRACE", "0") == "1"
    res = run_bass_kernel_spmd(nc, in_maps, list(range(NCORES)), trace=trace)
    LAST_EXEC_NS = res.exec_time_ns
    LAST_RESULTS = res

    out = np.empty((B, N, OUT), np.float32)
    for c in range(NCORES):
        out[c * BPC:(c + 1) * BPC] = res.results[c]["out"].reshape(BPC, N, OUT)
    return out


# revision 18
# speedup vs baseline: 1.0925x; 1.0925x over previous
"""Bass/Tile TRN2 kernel for nn_BatchGraphAttentionLayer (v2).

Reference computation (per batch b):
    Wh  = h[b] @ W                    # [64, 256]
    s1  = Wh @ a[:256], s2 = Wh @ a[256:]
    e   = leaky_relu(s1[i] + s2[j])   # [64, 64] (i rows, j cols)
    att = softmax over axis i of where(adj[i,j]>0, e, -9e15)
    out = elu(att @ Wh)               # h_prime[i] = sum_j att[i,j] Wh[j]

Sharding: data-parallel over batch, 8 cores x 4 batches.

v2 design (single bf16, measured absmax-rel err ~6e-3 vs 2e-2 gate):
  - a is folded into W on the host: W_ext = [W | W@a1 | W@a2] bf16
    [16384, 258].  The projection matmul then directly produces
    s1, s2 as PSUM columns 256:257 -- no separate score pass.
  - All h/W slab DMAs are issued up front into dedicated SBUF buffers
    (16 MiB total, fits).  No buffer reuse -> no DMA head-of-line
    stalls.  h streams on the sync HWDGE queue, W on the scalar one
    (~8 MiB each).  Slab sizes taper at both ends so the PE starts
    early and finishes shortly after the last byte lands.
  - Attention: batches are stacked 2 per 128-partition tile ("pairs");
    per pair, a rank-2 matmul builds the logit tile
        eT[j, i] = s1^t[i] + s2^t[j]
    from stationary S_t[2,128] = [ones; s2^t] and moving
    V_t[2,128] = [s1^t; ones], both derived from one PE transpose of
    the PSUM score columns via per-partition selector scalars.
  - softmax: exp(leaky_relu(e)) (logits are bounded, |s1+s2| < ~8, so
    unmasked exp is safe), masked by a 0/1 mask multiply, row-sum +
    reciprocal, scale.
  - out = elu(att @ Wh) with att/Wh in bf16 (error contribution
    negligible, keeps the PE in fast bf16 mode).
"""

import os
from contextlib import ExitStack

import ml_dtypes
import numpy as np

import concourse.bass as bass
import concourse.tile as tile
from concourse import bacc, mybir
from concourse.bass_utils import run_bass_kernel_spmd

F32 = mybir.dt.float32
BF16 = mybir.dt.bfloat16

B, N, IN, OUT = 32, 64, 16384, 256
OUTE = OUT + 2               # W_ext columns (W | Wa1 | Wa2)
NCORES = 8
BPC = B // NCORES            # batches per core = 4
M = BPC * N                  # local rows = 256
P = 128
ALPHA = 0.2
PRELU = os.environ.get("GAT_PRELU", "1") == "1"

KSUB = IN // P               # 128 k-subtiles of 128 rows
# taper both ends: fast PE ramp-up, short post-stream crunch
SLABS = [1, 1, 2, 4, 8, 16, 16, 16, 16, 16, 16, 8, 4, 2, 1, 1]
assert sum(SLABS) == KSUB

_NC = None
LAST_EXEC_NS = None
LAST_RESULTS = None


def _build_kernel(ctx: ExitStack, tc: tile.TileContext, out, h, w, mm128, eye):
    nc = tc.nc

    consts = ctx.enter_context(tc.tile_pool(name="consts", bufs=1))
    hpool = ctx.enter_context(tc.tile_pool(name="hslab", bufs=1))
    wpool = ctx.enter_context(tc.tile_pool(name="wslab", bufs=1))
    whpool = ctx.enter_context(tc.tile_pool(name="wh", bufs=1))
    attp = ctx.enter_context(tc.tile_pool(name="att", bufs=1))
    ps_accp = ctx.enter_context(tc.tile_pool(name="psacc", bufs=1, space="PSUM"))
    ps_smallp = ctx.enter_context(tc.tile_pool(name="pssmall", bufs=1, space="PSUM"))
    ps_ep = ctx.enter_context(tc.tile_pool(name="pse", bufs=1, space="PSUM"))
    ps_op = ctx.enter_context(tc.tile_pool(name="pso", bufs=1, space="PSUM"))

    # ---- constants (gpsimd/SWDGE keeps the HWDGE queues clear) ----
    sb_eye = consts.tile([P, P], F32)
    nc.gpsimd.dma_start(sb_eye, eye)
    sb_mm = consts.tile([P, P], F32)
    nc.gpsimd.dma_start(sb_mm, mm128)
    # ones row for the rank-1 e-matmul (partition 0 only: legal base)
    sb_ones1 = consts.tile([1, P], BF16)
    nc.vector.memset(sb_ones1, 1.0)

    # ---- stream DMAs: all issued up front, dedicated buffers ----
    hs = []
    ws = []
    k0 = 0
    for s, nsub in enumerate(SLABS):
        ksl = slice(k0 * P, (k0 + nsub) * P)
        ht = hpool.tile([P, nsub, M], BF16, tag=f"h{s}", name=f"h{s}")
        nc.sync.dma_start(ht, h[ksl, :].rearrange("(p c) m -> p c m", p=P))
        wt = wpool.tile([P, nsub, OUTE], BF16, tag=f"w{s}", name=f"w{s}")
        nc.scalar.dma_start(wt, w[ksl, :].rearrange("(p c) o -> p c o", p=P))
        hs.append(ht)
        ws.append(wt)
        k0 += nsub

    # ---- projection: Wh_ext accumulated in PSUM (m-major) ----
    ps_wh = [ps_accp.tile([P, OUTE], F32, tag=f"ps_wh{t}", name=f"ps_wh{t}")
             for t in range(2)]
    nslab = len(SLABS)
    for s, nsub in enumerate(SLABS):
        first = (s == 0)
        last = (s == nslab - 1)
        for c in range(nsub):
            for t in range(2):
                nc.tensor.matmul(ps_wh[t],
                                 lhsT=hs[s][:, c, t * P:(t + 1) * P],
                                 rhs=ws[s][:, c, :],
                                 start=(first and c == 0),
                                 stop=(last and c == nsub - 1),
                                 skip_group_check=True)

    # ---- scores: s1 columns -> rows via tiny PE transposes; s2 stays
    # a per-partition column and is folded into the leaky-relu bias ----
    # sc[:, 0:2] = (s1^0, s2^0), sc[:, 2:4] = (s1^1, s2^1)
    sc = attp.tile([P, 4], F32, tag="sc")
    nc.vector.tensor_copy(out=sc[:, 0:2], in_=ps_wh[0][:, OUT:OUTE])
    nc.scalar.copy(out=sc[:, 2:4], in_=ps_wh[1][:, OUT:OUTE])
    ps_e = [ps_ep.tile([P, P], F32, tag=f"ps_e{t}", name=f"ps_e{t}")
            for t in range(2)]
    lk = [attp.tile([P, P], F32, tag=f"lk{t}", name=f"lk{t}")
          for t in range(2)]
    for t in range(2):
        pst = ps_smallp.tile([2, P], F32, tag=f"pst{t}", name=f"pst{t}")
        nc.tensor.transpose(pst, sc[:, 2 * t:2 * t + 2], sb_eye)
        s1row = attp.tile([1, P], BF16, tag=f"s1row{t}")
        nc.vector.tensor_copy(out=s1row, in_=pst[0:1, :])
        # eT[j, i] = s1^t[i], replicated across j by the ones row
        nc.tensor.matmul(ps_e[t], lhsT=sb_ones1, rhs=s1row,
                         start=True, stop=True)
        s2col = sc[:, 2 * t + 1:2 * t + 2]
        if PRELU:
            # leaky(e + s2[j]) in one ACT op (bias is per-partition)
            nc.scalar.activation(lk[t], ps_e[t],
                                 mybir.ActivationFunctionType.Prelu,
                                 bias=s2col, alpha=ALPHA)
        else:
            va = attp.tile([P, P], F32, tag=f"va{t}")
            nc.vector.tensor_scalar(va, ps_e[t], s2col, None,
                                    mybir.AluOpType.add)
            vb = attp.tile([P, P], F32, tag=f"vb{t}")
            nc.vector.tensor_scalar(vb, ps_e[t], s2col, ALPHA,
                                    mybir.AluOpType.add,
                                    mybir.AluOpType.mult)
            nc.vector.tensor_tensor(lk[t], va, vb, mybir.AluOpType.max)

    # Wh to SBUF in bf16 (moving operand of the output matmul)
    wh_m = [whpool.tile([P, OUT], BF16, tag=f"wh_m{t}", name=f"wh_m{t}")
            for t in range(2)]
    nc.vector.tensor_copy(out=wh_m[0], in_=ps_wh[0][:, :OUT])
    nc.scalar.copy(out=wh_m[1], in_=ps_wh[1][:, :OUT])

    # ---- softmax: exp, fused mask-multiply + row-sum, scale ----
    rs = attp.tile([P, 2], F32, tag="rs")
    pexpm = [attp.tile([P, P], F32, tag=f"pexpm{t}", name=f"pexpm{t}")
             for t in range(2)]
    for t in range(2):
        pexp = attp.tile([P, P], F32, tag=f"pexp{t}")
        nc.scalar.activation(pexp, lk[t], mybir.ActivationFunctionType.Exp)
        nc.vector.tensor_tensor(pexpm[t], pexp, sb_mm, mybir.AluOpType.mult)
        nc.vector.tensor_reduce(rs[:, t:t + 1], pexpm[t],
                                axis=mybir.AxisListType.X,
                                op=mybir.AluOpType.add)
    rinv = attp.tile([P, 2], F32, tag="rinv")
    nc.vector.reciprocal(rinv, rs)
    att = attp.tile([P, 2 * P], BF16, tag="attw")
    nc.vector.tensor_scalar_mul(att[:, 0:P], pexpm[0], rinv[:, 0:1])
    nc.vector.tensor_scalar_mul(att[:, P:2 * P], pexpm[1], rinv[:, 1:2])

    # ---- out = elu(att @ Wh), pair-interleaved ----
    for t in range(2):
        ps_o = ps_op.tile([P, OUT], F32, tag=f"ps_o{t}", name=f"ps_o{t}")
        nc.tensor.matmul(ps_o, lhsT=att[:, t * P:(t + 1) * P], rhs=wh_m[t],
                         start=True, stop=True)
        # elu(x) = max(x,0) - 1 + exp(min(x,0))
        m0 = attp.tile([P, OUT], BF16, tag=f"m0{t}")
        nc.vector.tensor_scalar_min(m0, ps_o, 0.0)
        ex = attp.tile([P, OUT], F32, tag=f"ex{t}")
        nc.scalar.activation(ex, m0, mybir.ActivationFunctionType.Exp)
        rm1 = attp.tile([P, OUT], F32, tag=f"rm1{t}")
        nc.vector.tensor_scalar(rm1, ps_o, 0.0, -1.0,
                                mybir.AluOpType.max, mybir.AluOpType.add)
        ot = attp.tile([P, OUT], F32, tag=f"ot{t}")
        nc.vector.tensor_tensor(ot, ex, rm1, mybir.AluOpType.add)
        oeng = nc.sync if t == 0 else nc.scalar
        oeng.dma_start(out[t * P:(t + 1) * P, :], ot)


def _get_nc():
    global _NC
    if _NC is not None:
        return _NC
    nc = bacc.Bacc("TRN2", target_bir_lowering=False, debug=False,
                   num_devices=NCORES, disable_frame_to_traceback=True)
    h = nc.dram_tensor("h", [IN, M], BF16, kind="ExternalInput").ap()
    w = nc.dram_tensor("w", [IN, OUTE], BF16, kind="ExternalInput").ap()
    mm128 = nc.dram_tensor("mm128", [P, P], F32, kind="ExternalInput").ap()
    eye = nc.dram_tensor("eye", [P, P], F32, kind="ExternalInput").ap()
    out = nc.dram_tensor("out", [M, OUT], F32, kind="ExternalOutput").ap()
    with tile.TileContext(nc) as tc:
        with ExitStack() as ctx:
            _build_kernel(ctx, tc, out, h, w, mm128, eye)
    nc.compile()
    _NC = nc
    return nc


def kernel(h: np.ndarray, adj: np.ndarray, W: np.ndarray, a: np.ndarray
           ) -> np.ndarray:
    global LAST_EXEC_NS, LAST_RESULTS
    h = np.asarray(h, dtype=np.float32)
    W = np.asarray(W, dtype=np.float32)
    a = np.ascontiguousarray(np.asarray(a, dtype=np.float32)).reshape(2 * OUT)
    assert h.shape == (B, N, IN) and W.shape == (IN, OUT)

    nc = _get_nc()

    # mask [j~, i~]: pair-stacked adj^T on the block diagonal, both
    # column halves identical (one per pair)
    adjT = (np.asarray(adj) > 0).T.astype(np.float32)
    mm = np.zeros((P, P), np.float32)
    mm[:N, :N] = adjT
    mm[N:, N:] = adjT
    eye = np.eye(P, dtype=np.float32)

    Wa1 = (W.astype(np.float64) @ a[:OUT].astype(np.float64)).astype(np.float32)
    Wa2 = (W.astype(np.float64) @ a[OUT:].astype(np.float64)).astype(np.float32)
    w_ext = np.concatenate([W, Wa1[:, None], Wa2[:, None]], axis=1)
    w_ext = np.ascontiguousarray(w_ext).astype(ml_dtypes.bfloat16)

    in_maps = []
    for c in range(NCORES):
        hT = h[c * BPC:(c + 1) * BPC].reshape(M, IN).T
        in_maps.append({
            "h": np.ascontiguousarray(hT).astype(ml_dtypes.bfloat16),
            "w": w_ext, "mm128": mm, "eye": eye,
        })

    trace = os.environ.get("GAT_TRACE", "0") == "1"
    res = run_bass_kernel_spmd(nc, in_maps, list(range(NCORES)), trace=trace)
    LAST_EXEC_NS = res.exec_time_ns
    LAST_RESULTS = res

    out = np.empty((B, N, OUT), np.float32)
    for c in range(NCORES):
        out[c * BPC:(c + 1) * BPC] = res.results[c]["out"].reshape(BPC, N, OUT)
    return out
